# revision 19
# baseline (speedup 1.0000x reference)
"""DenseGATConv Bass/Tile kernel for Trainium2, SPMD over 8 NeuronCores.

Problem (B=4, N=2048, F=128, H=4, C=64):
  xh = (x @ W).reshape(B,N,H,C)
  a_src[b,j,h] = xh . att_src ; a_dst[b,i,h] = xh . att_dst
  s = a_src[j] + a_dst[i];  alpha = softmax_j(mask(adj+I, leaky_relu(s, 0.2)))
  out[b,i] = concat_h(sum_j alpha * xh[b,j,h,:]) + bias

Algebra (no exp over the N*N*H grid):
  exp(lrelu(s))/exp(a_dst_i) = max(e2_j, Q'_i e1_j)
      e1 = exp(.2 a_src), e2 = exp(a_src), Q' = exp(-.8 a_dst)
  Grid weight G = adjT * max(Q' e1, e2); PE accumulates
      acc[h][c,i] += xh1[j, c|1]^T @ G[j, h, i]   (fp16, f32 PSUM)
  with a ones column so row 64 of acc is the softmax denominator.

Engine split per source tile (DVE saturates otherwise):
  heads 0-2: T = tensor_scalar(mult,max) on DVE + fused rep-3 mask mult (DVE)
  head 3:    R = relu(e1 Q' - e2) on ACT (max(a,b) = relu(a-b)+b), mask mult
             on GPSIMD, and the +e2 branch restored exactly on the PE via
             acc[3] += [e2*xh3 | e2]^T @ adjT.
  a_dst is folded into the projection (wcat cols 264:268 = -.8 W att_dst);
  a small dedicated 4-column matmul per dest tile produces Q' early, bounced
  through DRAM into a per-head partition-broadcast q_bc (dest order matches
  the adjT column permutation).

Sharding: core = b*2 + ihalf; each core owns 1024 destination rows of one
batch and reads that batch's full source side (adj slice pre-transposed,
self-loops added, fp16-cast, source tiles host-reordered dest-half-first).
"""

import os

import numpy as np

import concourse.bacc as bacc
import concourse.bass as bass
import concourse.dve_ops as dve_ops_mod
import concourse.tile as tile
from concourse import mybir
from concourse.bass_utils import run_bass_kernel_spmd
from concourse.dve_spec import C0 as _C0, C1 as _C1, Spec as _Spec, \
    Src0 as _Src0, Src1 as _Src1, lower as _dve_lower, maxx as _maxx
from concourse.dve_uop import (
    AluInp as _AluInp,
    AluOp as _AluOp,
    DelayInp as _DelayInp,
    DveOpSpec as _DveOpSpec,
    InpSel as _InpSel,
    OutPath as _OutPath,
    OutSel as _OutSel,
    UopConfig as _UopConfig,
)
from concourse.masks import make_identity

B, N, F = 4, 2048, 128
H, C = 4, 64
HC = H * C
N_CORES = 8
ID = N // 2          # dest rows per core
NT = N // 128        # 16 source tiles
NKD = ID // 512      # 2 dest 512-chunks
NDT = NT // 2        # 8 dest-half tiles (projected first)
F32 = mybir.dt.float32
F16 = mybir.dt.float16

TBUFS = int(os.environ.get('TBUFS', 4))
GBUFS = int(os.environ.get('GBUFS', 6))
ABUFS = int(os.environ.get('ABUFS', 3))

_NC_CACHE = {}

# ---------------------------------------------------------------------------
# Custom fused DVE op: out = max(in0*s0, s1) * in1 in ONE Vector instruction,
# with a hand-written 2X_1PORT microcode variant (fp16 packed pairs).  The
# stock path needs tensor_scalar + tensor_tensor — two full passes over the
# N*N*H grid; this op does the whole per-element GAT weight in one pass at
# 2 elements/cycle.  Registered into concourse.dve_ops' tables at import
# (that file's OPS list is the designed extension point; the repo is
# read-only here so we append at runtime before any kernel is built).
# ---------------------------------------------------------------------------
_MAXMUL_NAME = "MAXMUL_2X_ANT"


def _maxmul_2x_uop(base: "_UopConfig") -> "_UopConfig":
    """2X_1PORT program: lo/hi packed-fp16 chains in parallel, mirroring the
    stock gen3 tensor_scalar perf-mode microcode idiom (results ride delay
    chains d2/d3 to the write mux)."""
    u = _UopConfig()
    u.trigger = base.trigger
    u.next_uop = base.next_uop
    u.repeat_count = base.repeat_count
    u.require_inp0 = base.require_inp0
    u.require_inp1 = base.require_inp1

    u.enable_input(_InpSel.SRC_0, 0)       # block0 PREV_ALU_OUT
    u.enable_input(_InpSel.CONST_0, 1)     # PD0
    u.enable_input(_InpSel.CONST_1, 2)     # PD1
    u.enable_input(_InpSel.SRC_0_HI, 3)    # PD2
    u.enable_input(_InpSel.SRC_1, 4)       # PD3
    u.enable_input(_InpSel.SRC_1_HI, 5)    # PD4

    b = u.datapath_config
    # blk0: u_lo = Src0_lo * C0
    b[0].enable_alu(_AluOp.MULTIPLY, _AluInp.PREV_ALU_OUT, _AluInp.PREV_DELAY_0)
    b[0].pass_through_delay(0, 1, 2, 3, 4)
    # blk1: u_hi = Src0_hi * C0 ; d5 <- u_lo
    b[1].enable_alu(_AluOp.MULTIPLY, _AluInp.PREV_DELAY_2, _AluInp.PREV_DELAY_0)
    b[1].enable_delay_from_src(_DelayInp.PREV_ALU_OUT, 5)
    b[1].pass_through_delay(1, 3, 4)
    # blk2: v_lo = max(u_lo, C1) ; d2 <- u_hi
    b[2].enable_alu(_AluOp.MAX, _AluInp.PREV_DELAY_5, _AluInp.PREV_DELAY_1)
    b[2].enable_delay_from_src(_DelayInp.PREV_ALU_OUT, 2)
    b[2].pass_through_delay(1, 3, 4)
    # blk3: v_hi = max(u_hi, C1) ; d5 <- v_lo
    b[3].enable_alu(_AluOp.MAX, _AluInp.PREV_DELAY_2, _AluInp.PREV_DELAY_1)
    b[3].enable_delay_from_src(_DelayInp.PREV_ALU_OUT, 5)
    b[3].pass_through_delay(3, 4)
    # blk4: w_lo = v_lo * Src1_lo ; d2 <- v_hi
    b[4].enable_alu(_AluOp.MULTIPLY, _AluInp.PREV_DELAY_5, _AluInp.PREV_DELAY_3)
    b[4].enable_delay_from_src(_DelayInp.PREV_ALU_OUT, 2)
    b[4].pass_through_delay(4)
    # blk5: w_hi = v_hi * Src1_hi ; d2 <- w_lo
    b[5].enable_alu(_AluOp.MULTIPLY, _AluInp.PREV_DELAY_2, _AluInp.PREV_DELAY_4)
    b[5].enable_delay_from_src(_DelayInp.PREV_ALU_OUT, 2)
    # blk6: d2 passes w_lo, d3 <- w_hi ; blk7 carries both to the write mux
    b[6].pass_through_delay(2)
    b[6].enable_delay_from_src(_DelayInp.PREV_ALU_OUT, 3)
    b[7].pass_through_delay(2, 3)

    u.enable_output(_OutSel.DELAY_2, _OutPath.WR0_LO)
    u.enable_output(_OutSel.DELAY_3, _OutPath.WR0_HI)
    return u


class _MaxMulOp:
    name = _MAXMUL_NAME
    subdim = False
    spec = _Spec(
        body=_maxx(_Src0 * _C0, _C1) * _Src1,
        reference=lambda in0, in1, s0, s1, imm2: (
            np.maximum(in0.astype(np.float32) * s0, s1) * in1
        ),
    )

    def __init__(self):
        self._cache = {}

    def compile(self, ver):
        if ver in self._cache:
            return self._cache[ver]
        uops = _dve_lower(self.spec, ver=ver)
        assert len(uops) == 1
        spec = _DveOpSpec(
            name=self.name,
            opcode=dve_ops_mod.get_dve_sub_opcode(self.name),
            uops=uops,
            uops_2x=[_maxmul_2x_uop(uops[0])],
            perf_max=1,
            rd1_en=True,
        )
        for u in spec.uops + spec.uops_2x:
            u.validate(ver)
        self._cache[ver] = spec
        return spec


_MAXMUL_OP = _MaxMulOp()


def _register_maxmul():
    if _MAXMUL_NAME in dve_ops_mod._SUB_OPCODE_FOR_NAME:
        return
    row = max(dve_ops_mod._SUB_OPCODE_FOR_NAME.values()) + 1
    assert row < 0x20
    dve_ops_mod._SUB_OPCODE_FOR_NAME[_MAXMUL_NAME] = row
    dve_ops_mod.OPS.append(_MAXMUL_OP)
    dve_ops_mod.CUSTOM_DVE_SPECS[_MAXMUL_NAME] = _MAXMUL_OP.spec


def _maxmul(nc, *, out, in0, in1, s0, s1):
    """out = max(in0*s0, s1) * in1 — one 2x-rate DVE instruction."""
    inst = nc.vector._custom_dve(
        _MAXMUL_OP, out=out, in0=in0, in1=in1, s0=s0, s1=s1)
    # deployed firmware keys the 2x path on byte-36 bit 7 (perf_max=2):
    # measured 3580 ns/op with perf_max=1, 844 ns/op with perf_max=2
    inst.ins.perf_max = 2
    return inst


def build_nc(reps: int = 1):
    _register_maxmul()
    nc = bacc.Bacc("TRN2", target_bir_lowering=False, debug=False, num_devices=1)

    d_xT = nc.dram_tensor("xT", [F, N], F16, kind="ExternalInput").ap()
    d_adjT = nc.dram_tensor("adjT", [NT, 128, ID], F16, kind="ExternalInput").ap()
    d_wcat = nc.dram_tensor("Wcat", [F, HC + 12], F16, kind="ExternalInput").ap()
    d_bias = nc.dram_tensor("biasv", [1, HC], F32, kind="ExternalInput").ap()
    d_out = nc.dram_tensor("out", [ID, HC], F32, kind="ExternalOutput").ap()

    EXP = mybir.ActivationFunctionType.Exp
    CPY = mybir.ActivationFunctionType.Copy
    RELU = mybir.ActivationFunctionType.Relu
    MULT = mybir.AluOpType.mult
    MAX = mybir.AluOpType.max
    ADD = mybir.AluOpType.add

    with tile.TileContext(nc) as tc:
        with tc.tile_pool(name="const", bufs=1) as const:
            # --- inputs first: wq + the dest-half xT chunks gate the q chain
            wq = const.tile([F, 4], F16)
            nc.scalar.dma_start(out=wq, in_=d_wcat[:, HC + 8:HC + 12])
            xT = const.tile([F, N], F16)
            for lo, hi in ((0, ID), (ID, ID + 512), (ID + 512, N)):
                nc.gpsimd.dma_start(out=xT[:, lo:hi], in_=d_xT[:, lo:hi])
            wcat = const.tile([F, HC + 12], F16)
            nc.scalar.dma_start(out=wcat, in_=d_wcat)
            bias_sb = const.tile([1, HC], F32)
            nc.scalar.dma_start(out=bias_sb, in_=d_bias)

            ident = const.tile([128, 128], F32)
            make_identity(nc, ident)
            ones1 = const.tile([1, 128], F32)
            nc.vector.memset(ones1, 1.0)
            # preload the exp table set while input DMAs run
            scratch1 = const.tile([1, 4], F32)
            nc.scalar.activation(scratch1, ones1[0:1, 0:4], EXP)

            # persistent per-core tensors
            xh1 = const.tile([128, NT, H, 65], F16)     # [xh | 1] per (t,h)
            expv = const.tile([128, NT, 8], F32)        # e1 (4) | e2 (4)
            qrow4 = const.tile([H, ID], F16)            # exp(-.8 a_dst) rows
            mmat = const.tile([65, H, 65], F32)         # bias-augmented identity
            q_bc = const.tile([128, H, ID], F16)        # Q' broadcast per head
            bias_bc = const.tile([128, HC], F32)

            nc.gpsimd.memset(xh1[:, :, :, 64:65], 1.0)

            # ---------------- phase A: projections ----------------
            sc_a = nc.enter_named_scope("phA", False)
            with tc.tile_pool(name="psA", bufs=2, space="PSUM") as psA, \
                 tc.tile_pool(name="psQ", bufs=2, space="PSUM") as psQ, \
                 tc.tile_pool(name="psB", bufs=1, space="PSUM") as psBp:
                # q projection flipped: stationary = the 4 -.8 W att_dst
                # cols, moving = the (perm-ordered) dest-half xT columns.
                # Yields a_dst^T as [4, 1024] rows directly -- no transpose,
                # no per-tile chain; one EXP then a DRAM-bounce broadcast
                # (per head, on the scalar hwdge queue -- the sync queue
                # carries the adjT stream).
                psq = psQ.tile([4, ID], F32, tag="psq", bufs=1)
                for kb in range(NKD):
                    nc.tensor.matmul(psq[:, kb * 512:(kb + 1) * 512], wq,
                                     xT[:, kb * 512:(kb + 1) * 512],
                                     start=True, stop=True)
                nc.scalar.activation(qrow4, psq, EXP)
                with tc.tile_pool(name="dscr", bufs=1, space="DRAM") as dscr:
                    qscr = dscr.tile([H, ID], F16)
                    nc.scalar.dma_start(out=qscr, in_=qrow4)
                    for h in range(H):
                        qrow = qscr[h:h + 1, :]
                        src_h = bass.AP(
                            tensor=qrow.tensor, offset=qrow.offset,
                            ap=[[0, 128], [1, ID]])
                        nc.scalar.dma_start(out=q_bc[:, h, :], in_=src_h)

                # bias broadcast (epilogue-only)
                psb2 = psBp.tile([128, HC], F32, tag="psbias", bufs=1)
                nc.tensor.matmul(psb2, ones1, bias_sb, start=True, stop=True)
                nc.scalar.activation(bias_bc, psb2, CPY)
                # bias-augmented transpose matrices: M_h = I65 with row 64 =
                # [bias_h | 1].  pt = s^T @ M_h then equals num + den*bias in
                # cols 0:64 (and den in col 64), so after the reciprocal
                # multiply the bias is already applied -- no separate add.
                idsl = ident[0:65, 0:65]
                ident_b = bass.AP(
                    tensor=idsl.tensor, offset=idsl.offset,
                    ap=[idsl.ap[0], [0, H], [1, 65]])
                nc.vector.tensor_copy(mmat, ident_b)
                nc.scalar.activation(mmat[64:65, :, 0:64],
                                     bias_bc[64:65, :], CPY)
                nc.gpsimd.memset(mmat[64:65, :, 64:65], 1.0)

                # main projections; expv exp + xh1 evac on ACT
                for t in range(NT):
                    ps = psA.tile([128, HC + 12], F32)
                    nc.tensor.matmul(ps, xT[:, t * 128:(t + 1) * 128], wcat,
                                     start=True, stop=True)
                    nc.scalar.activation(expv[:, t, :], ps[:, HC:HC + 8], EXP)
                    nc.scalar.activation(xh1[:, t, :, 0:64], ps[:, 0:HC], CPY)

            nc.leave_named_scope("phA", sc_a[0], False)

            # ---------------- phase B: grid + matmul accumulate ----------------
            with tc.tile_pool(name="ep_sb", bufs=1) as epsb:
                with tc.tile_pool(name="acc", bufs=1, space="PSUM") as accp:
                    acc = {}
                    for h in range(H):
                        acc[h] = accp.tile([65, ID], F32, tag=f"acc{h}",
                                           name=f"acc{h}")

                    sc_b = nc.enter_named_scope("phB", False)
                    with tc.tile_pool(name="adj", bufs=ABUFS) as adjp, \
                         tc.tile_pool(name="grid", bufs=4) as gridp:
                        for rep in range(reps):
                            for t in range(NT):
                                adjt = adjp.tile([128, ID], F16)
                                nc.sync.dma_start(out=adjt, in_=d_adjT[t])
                                g = gridp.tile([128, H, ID], F16, tag="G",
                                               bufs=GBUFS)
                                # fused grid weight, one 2x DVE op per head:
                                # g = max(Q' e1, e2) * adjT
                                for h in range(H):
                                    _maxmul(
                                        nc, out=g[:, h, :], in0=q_bc[:, h, :],
                                        in1=adjt,
                                        s0=expv[:, t, h:h + 1],
                                        s1=expv[:, t, 4 + h:5 + h])

                                first = (rep == 0 and t == 0)
                                last = (rep == reps - 1 and t == NT - 1)
                                for h in range(H):
                                    for k in range(NKD):
                                        nc.tensor.matmul(
                                            acc[h][:, k * 512:(k + 1) * 512],
                                            xh1[:, t, h, :],
                                            g[:, h, k * 512:(k + 1) * 512],
                                            start=first, stop=last)

                    nc.leave_named_scope("phB", sc_b[0], False)
                    sc_c = nc.enter_named_scope("phC", False)
                    # evacuate accumulators to SBUF, split ACT/DVE, per (h,k)
                    s_tiles = {}
                    for k in range(NKD):
                        for h in range(H):
                            s = epsb.tile([65, 512], F32, tag=f"s{h}{k}",
                                          name=f"s{h}{k}")
                            sl = acc[h][:, k * 512:(k + 1) * 512]
                            # acc0/acc1 evacuate on ACT, acc2/acc3 on DVE so
                            # the low PSUM banks (reused by the transpose
                            # tiles) free earliest
                            if h < 2:
                                nc.scalar.activation(s, sl, CPY)
                            else:
                                nc.vector.tensor_copy(s, sl)
                            s_tiles[(h, k)] = s

                # acc PSUM released here
                # ------------- phase C: transpose + divide + bias + out -------------
                with tc.tile_pool(name="ep_ps", bufs=1, space="PSUM") as epps, \
                     tc.tile_pool(name="ep_sm", bufs=2) as epsm, \
                     tc.tile_pool(name="outp", bufs=2) as outp:
                    for k in range(NKD):
                        # all four heads transposed into one PSUM tile,
                        # 128-padded blocks so no transpose crosses a bank
                        pt = epps.tile([128, H, 4, 128], F32)
                        for h in range(H):
                            for kk in range(4):
                                nc.tensor.transpose(
                                    pt[:, h, kk, 0:65],
                                    s_tiles[(h, k)][:, kk * 128:(kk + 1) * 128],
                                    mmat[:, h, :])
                        rec = epsm.tile([128, H, 4, 1], F32)
                        nc.vector.reciprocal(rec, pt[:, :, :, 64:65])
                        osb = outp.tile([128, 4, HC], F32, tag="osb", name="osb")
                        rec_rep = bass.AP(
                            tensor=rec.tensor, offset=rec.offset,
                            ap=[rec.ap[0], rec.ap[1], rec.ap[2], [0, 64]])
                        # osb[p, kk, h*64+c] = pt[p, h, kk, c] * rec[p, h, kk]
                        osb_x = bass.AP(
                            tensor=osb.tensor, offset=osb.offset,
                            ap=[osb.ap[0], [64, H], [HC, 4], [1, 64]])
                        nc.vector.tensor_tensor(
                            out=osb_x, in0=pt[:, :, :, 0:64], in1=rec_rep,
                            op=MULT)
                        # destination rows are host-permuted so partition p
                        # holds 4 consecutive output rows: one contiguous 4KB
                        # descriptor per partition
                        # two half-DMAs on different queues so the tail
                        # transfer overlaps
                        blk = d_out[k * 512:(k + 1) * 512, :]
                        for pp, eng in ((0, nc.sync), (1, nc.scalar)):
                            sub = bass.AP(
                                tensor=blk.tensor,
                                offset=blk.offset + pp * 64 * 4 * HC,
                                ap=[[4 * HC, 64], [HC, 4], [1, HC]])
                            eng.dma_start(out=sub, in_=osb[pp * 64:(pp + 1) * 64])
                    nc.leave_named_scope("phC", sc_c[0], False)

    nc.compile()
    return nc


def _get_nc(reps: int = 1):
    if reps not in _NC_CACHE:
        _NC_CACHE[reps] = build_nc(reps)
    return _NC_CACHE[reps]


def make_in_maps(x, adj, W, att_src, att_dst, bias):
    x = np.asarray(x, dtype=np.float32)
    adj = np.asarray(adj, dtype=np.float32)
    W = np.asarray(W, dtype=np.float32)
    att_src = np.asarray(att_src, dtype=np.float32)
    att_dst = np.asarray(att_dst, dtype=np.float32)
    bias = np.asarray(bias, dtype=np.float32)

    # weight prep: fold per-head attention dots into projection columns
    wa_src = np.stack([W[:, h * C:(h + 1) * C] @ att_src[h] for h in range(H)], 1)
    wa_dst = np.stack([W[:, h * C:(h + 1) * C] @ att_dst[h] for h in range(H)], 1)
    wcat = np.concatenate([W, 0.2 * wa_src, wa_src, -0.8 * wa_dst], axis=1)
    wcat = np.ascontiguousarray(wcat, dtype=np.float16)          # [F, 268]

    adjl = adj.copy()
    idx = np.arange(N)
    adjl[:, idx, idx] = 1.0

    # destination-row permutation: kernel position i' = kk*128 + p within each
    # 512-block maps to original row p*4 + kk, so the output DMA writes 4KB
    # contiguous chunks per partition
    perm = np.concatenate([kb * 512 + (np.arange(512) % 128) * 4 + np.arange(512) // 128
                           for kb in range(ID // 512)])

    in_maps = []
    for c in range(N_CORES):
        b, half = c // 2, c % 2
        # kernel source-row order: dest-half rows first, in the dest
        # permutation (so q values line up with adjT columns), then the rest
        row_order = np.concatenate([half * ID + perm,
                                    (1 - half) * ID + np.arange(ID)])
        xT = np.ascontiguousarray(x[b][row_order].T, dtype=np.float16)
        adjT = np.ascontiguousarray(
            adjl[b].T[row_order][:, half * ID:(half + 1) * ID][:, perm]
        ).astype(np.float16)       # [j(reordered), i(permuted)]
        in_maps.append({
            "xT": xT,
            "adjT": adjT.reshape(NT, 128, ID),
            "Wcat": wcat,
            "biasv": bias.reshape(1, HC),
        })
    return in_maps


def assemble(results):
    out = np.empty((B, N, HC), dtype=np.float32)
    for c in range(N_CORES):
        b, half = c // 2, c % 2
        out[b, half * ID:(half + 1) * ID, :] = results[c]["out"]
    return out


def kernel(x, adj, W, att_src, att_dst, bias):
    nc = _get_nc(1)
    in_maps = make_in_maps(x, adj, W, att_src, att_dst, bias)
    res = run_bass_kernel_spmd(nc, in_maps, list(range(N_CORES)))
    return assemble(res.results)


# revision 20
# speedup vs baseline: 1.0183x; 1.0183x over previous
"""DenseGATConv Bass/Tile kernel for Trainium2, SPMD over 8 NeuronCores.

Problem (B=4, N=2048, F=128, H=4, C=64):
  xh = (x @ W).reshape(B,N,H,C)
  a_src[b,j,h] = xh . att_src ; a_dst[b,i,h] = xh . att_dst
  s = a_src[j] + a_dst[i];  alpha = softmax_j(mask(adj+I, leaky_relu(s, 0.2)))
  out[b,i] = concat_h(sum_j alpha * xh[b,j,h,:]) + bias

Algebra (no exp over the N*N*H grid):
  exp(lrelu(s))/exp(a_dst_i) = max(e2_j, Q'_i e1_j)
      e1 = exp(.2 a_src), e2 = exp(a_src), Q' = exp(-.8 a_dst)
  Grid weight G = adjT * max(Q' e1, e2); PE accumulates
      acc[h][c,i] += xh1[j, c|1]^T @ G[j, h, i]   (fp16, f32 PSUM)
  with a ones column so row 64 of acc is the softmax denominator.

Engine split per source tile (DVE saturates otherwise):
  heads 0-2: T = tensor_scalar(mult,max) on DVE + fused rep-3 mask mult (DVE)
  head 3:    R = relu(e1 Q' - e2) on ACT (max(a,b) = relu(a-b)+b), mask mult
             on GPSIMD, and the +e2 branch restored exactly on the PE via
             acc[3] += [e2*xh3 | e2]^T @ adjT.
  a_dst is folded into the projection (wcat cols 264:268 = -.8 W att_dst);
  a small dedicated 4-column matmul per dest tile produces Q' early, bounced
  through DRAM into a per-head partition-broadcast q_bc (dest order matches
  the adjT column permutation).

Sharding: core = b*2 + ihalf; each core owns 1024 destination rows of one
batch and reads that batch's full source side (adj slice pre-transposed,
self-loops added, fp16-cast, source tiles host-reordered dest-half-first).
"""

import os

import numpy as np

import concourse.bacc as bacc
import concourse.bass as bass
import concourse.dve_ops as dve_ops_mod
import concourse.tile as tile
from concourse import mybir
from concourse.bass_utils import run_bass_kernel_spmd
from concourse.dve_spec import C0 as _C0, C1 as _C1, Spec as _Spec, \
    Src0 as _Src0, Src1 as _Src1, lower as _dve_lower, maxx as _maxx
from concourse.dve_uop import (
    AluInp as _AluInp,
    AluOp as _AluOp,
    DelayInp as _DelayInp,
    DveOpSpec as _DveOpSpec,
    InpSel as _InpSel,
    OutPath as _OutPath,
    OutSel as _OutSel,
    UopConfig as _UopConfig,
)
from concourse.masks import make_identity

B, N, F = 4, 2048, 128
H, C = 4, 64
HC = H * C
N_CORES = 8
ID = N // 2          # dest rows per core
NT = N // 128        # 16 source tiles
NKD = ID // 512      # 2 dest 512-chunks
NDT = NT // 2        # 8 dest-half tiles (projected first)
F32 = mybir.dt.float32
F16 = mybir.dt.float16

TBUFS = int(os.environ.get('TBUFS', 4))
GBUFS = int(os.environ.get('GBUFS', 6))
ABUFS = int(os.environ.get('ABUFS', 3))

_NC_CACHE = {}

# ---------------------------------------------------------------------------
# Custom fused DVE op: out = max(in0*s0, s1) * in1 in ONE Vector instruction,
# with a hand-written 2X_1PORT microcode variant (fp16 packed pairs).  The
# stock path needs tensor_scalar + tensor_tensor — two full passes over the
# N*N*H grid; this op does the whole per-element GAT weight in one pass at
# 2 elements/cycle.  Registered into concourse.dve_ops' tables at import
# (that file's OPS list is the designed extension point; the repo is
# read-only here so we append at runtime before any kernel is built).
# ---------------------------------------------------------------------------
_MAXMUL_NAME = "MAXMUL_2X_ANT"


def _maxmul_2x_uop(base: "_UopConfig") -> "_UopConfig":
    """2X_1PORT program: lo/hi packed-fp16 chains in parallel, mirroring the
    stock gen3 tensor_scalar perf-mode microcode idiom (results ride delay
    chains d2/d3 to the write mux)."""
    u = _UopConfig()
    u.trigger = base.trigger
    u.next_uop = base.next_uop
    u.repeat_count = base.repeat_count
    u.require_inp0 = base.require_inp0
    u.require_inp1 = base.require_inp1

    u.enable_input(_InpSel.SRC_0, 0)       # block0 PREV_ALU_OUT
    u.enable_input(_InpSel.CONST_0, 1)     # PD0
    u.enable_input(_InpSel.CONST_1, 2)     # PD1
    u.enable_input(_InpSel.SRC_0_HI, 3)    # PD2
    u.enable_input(_InpSel.SRC_1, 4)       # PD3
    u.enable_input(_InpSel.SRC_1_HI, 5)    # PD4

    b = u.datapath_config
    # blk0: u_lo = Src0_lo * C0
    b[0].enable_alu(_AluOp.MULTIPLY, _AluInp.PREV_ALU_OUT, _AluInp.PREV_DELAY_0)
    b[0].pass_through_delay(0, 1, 2, 3, 4)
    # blk1: u_hi = Src0_hi * C0 ; d5 <- u_lo
    b[1].enable_alu(_AluOp.MULTIPLY, _AluInp.PREV_DELAY_2, _AluInp.PREV_DELAY_0)
    b[1].enable_delay_from_src(_DelayInp.PREV_ALU_OUT, 5)
    b[1].pass_through_delay(1, 3, 4)
    # blk2: v_lo = max(u_lo, C1) ; d2 <- u_hi
    b[2].enable_alu(_AluOp.MAX, _AluInp.PREV_DELAY_5, _AluInp.PREV_DELAY_1)
    b[2].enable_delay_from_src(_DelayInp.PREV_ALU_OUT, 2)
    b[2].pass_through_delay(1, 3, 4)
    # blk3: v_hi = max(u_hi, C1) ; d5 <- v_lo
    b[3].enable_alu(_AluOp.MAX, _AluInp.PREV_DELAY_2, _AluInp.PREV_DELAY_1)
    b[3].enable_delay_from_src(_DelayInp.PREV_ALU_OUT, 5)
    b[3].pass_through_delay(3, 4)
    # blk4: w_lo = v_lo * Src1_lo ; d2 <- v_hi
    b[4].enable_alu(_AluOp.MULTIPLY, _AluInp.PREV_DELAY_5, _AluInp.PREV_DELAY_3)
    b[4].enable_delay_from_src(_DelayInp.PREV_ALU_OUT, 2)
    b[4].pass_through_delay(4)
    # blk5: w_hi = v_hi * Src1_hi ; d2 <- w_lo
    b[5].enable_alu(_AluOp.MULTIPLY, _AluInp.PREV_DELAY_2, _AluInp.PREV_DELAY_4)
    b[5].enable_delay_from_src(_DelayInp.PREV_ALU_OUT, 2)
    # blk6: d2 passes w_lo, d3 <- w_hi ; blk7 carries both to the write mux
    b[6].pass_through_delay(2)
    b[6].enable_delay_from_src(_DelayInp.PREV_ALU_OUT, 3)
    b[7].pass_through_delay(2, 3)

    u.enable_output(_OutSel.DELAY_2, _OutPath.WR0_LO)
    u.enable_output(_OutSel.DELAY_3, _OutPath.WR0_HI)
    return u


class _MaxMulOp:
    name = _MAXMUL_NAME
    subdim = False
    spec = _Spec(
        body=_maxx(_Src0 * _C0, _C1) * _Src1,
        reference=lambda in0, in1, s0, s1, imm2: (
            np.maximum(in0.astype(np.float32) * s0, s1) * in1
        ),
    )

    def __init__(self):
        self._cache = {}

    def compile(self, ver):
        if ver in self._cache:
            return self._cache[ver]
        uops = _dve_lower(self.spec, ver=ver)
        assert len(uops) == 1
        spec = _DveOpSpec(
            name=self.name,
            opcode=dve_ops_mod.get_dve_sub_opcode(self.name),
            uops=uops,
            uops_2x=[_maxmul_2x_uop(uops[0])],
            perf_max=1,
            rd1_en=True,
        )
        for u in spec.uops + spec.uops_2x:
            u.validate(ver)
        self._cache[ver] = spec
        return spec


_MAXMUL_OP = _MaxMulOp()


def _register_maxmul():
    if _MAXMUL_NAME in dve_ops_mod._SUB_OPCODE_FOR_NAME:
        return
    row = max(dve_ops_mod._SUB_OPCODE_FOR_NAME.values()) + 1
    assert row < 0x20
    dve_ops_mod._SUB_OPCODE_FOR_NAME[_MAXMUL_NAME] = row
    dve_ops_mod.OPS.append(_MAXMUL_OP)
    dve_ops_mod.CUSTOM_DVE_SPECS[_MAXMUL_NAME] = _MAXMUL_OP.spec


def _maxmul(nc, *, out, in0, in1, s0, s1):
    """out = max(in0*s0, s1) * in1 — one 2x-rate DVE instruction."""
    inst = nc.vector._custom_dve(
        _MAXMUL_OP, out=out, in0=in0, in1=in1, s0=s0, s1=s1)
    # deployed firmware keys the 2x path on byte-36 bit 7 (perf_max=2):
    # measured 3580 ns/op with perf_max=1, 844 ns/op with perf_max=2
    inst.ins.perf_max = 2
    return inst


def build_nc(reps: int = 1):
    _register_maxmul()
    nc = bacc.Bacc("TRN2", target_bir_lowering=False, debug=False, num_devices=1)

    d_xT = nc.dram_tensor("xT", [F, N], F16, kind="ExternalInput").ap()
    d_adjT = nc.dram_tensor("adjT", [NT, 128, ID], F16, kind="ExternalInput").ap()
    d_wcat = nc.dram_tensor("Wcat", [F, HC + 12], F16, kind="ExternalInput").ap()
    d_bias = nc.dram_tensor("biasv", [1, HC], F32, kind="ExternalInput").ap()
    d_out = nc.dram_tensor("out", [ID, HC], F32, kind="ExternalOutput").ap()

    EXP = mybir.ActivationFunctionType.Exp
    CPY = mybir.ActivationFunctionType.Copy
    RELU = mybir.ActivationFunctionType.Relu
    MULT = mybir.AluOpType.mult
    MAX = mybir.AluOpType.max
    ADD = mybir.AluOpType.add

    with tile.TileContext(nc) as tc:
        with tc.tile_pool(name="const", bufs=1) as const:
            # --- inputs first: wq + the dest-half xT chunks gate the q chain
            wq = const.tile([F, 4], F16)
            nc.scalar.dma_start(out=wq, in_=d_wcat[:, HC + 8:HC + 12])
            xT = const.tile([F, N], F16)
            for lo, hi in ((0, ID), (ID, ID + 512), (ID + 512, N)):
                nc.gpsimd.dma_start(out=xT[:, lo:hi], in_=d_xT[:, lo:hi])
            wcat = const.tile([F, HC + 12], F16)
            nc.scalar.dma_start(out=wcat, in_=d_wcat)
            bias_sb = const.tile([1, HC], F32)
            nc.scalar.dma_start(out=bias_sb, in_=d_bias)

            ident = const.tile([128, 128], F32)
            make_identity(nc, ident)
            ones1 = const.tile([1, 128], F32)
            nc.vector.memset(ones1, 1.0)
            # preload the exp table set while input DMAs run
            scratch1 = const.tile([1, 4], F32)
            nc.scalar.activation(scratch1, ones1[0:1, 0:4], EXP)

            # persistent per-core tensors
            xh1 = const.tile([128, NT, H, 65], F16)     # [xh | 1] per (t,h)
            expv = const.tile([128, NT, 8], F32)        # e1 (4) | e2 (4)
            qrow4 = const.tile([H, ID], F16)            # exp(-.8 a_dst) rows
            mmat = const.tile([65, H, 65], F32)         # bias-augmented identity
            q_bc = const.tile([128, H, ID], F16)        # Q' broadcast per head
            bias_bc = const.tile([128, HC], F32)

            nc.gpsimd.memset(xh1[:, :, :, 64:65], 1.0)

            # ---------------- phase A: projections ----------------
            sc_a = nc.enter_named_scope("phA", False)
            with tc.tile_pool(name="psA", bufs=2, space="PSUM") as psA, \
                 tc.tile_pool(name="psQ", bufs=2, space="PSUM") as psQ, \
                 tc.tile_pool(name="psB", bufs=1, space="PSUM") as psBp:
                # q projection flipped: stationary = the 4 -.8 W att_dst
                # cols, moving = the (perm-ordered) dest-half xT columns.
                # Yields a_dst^T as [4, 1024] rows directly -- no transpose,
                # no per-tile chain; one EXP then a DRAM-bounce broadcast
                # (per head, on the scalar hwdge queue -- the sync queue
                # carries the adjT stream).
                psq = psQ.tile([4, ID], F32, tag="psq", bufs=1)
                for kb in range(NKD):
                    nc.tensor.matmul(psq[:, kb * 512:(kb + 1) * 512], wq,
                                     xT[:, kb * 512:(kb + 1) * 512],
                                     start=True, stop=True)
                nc.scalar.activation(qrow4, psq, EXP)
                with tc.tile_pool(name="dscr", bufs=1, space="DRAM") as dscr:
                    qscr = dscr.tile([H, ID], F16)
                    nc.scalar.dma_start(out=qscr, in_=qrow4)
                    for h in range(H):
                        qrow = qscr[h:h + 1, :]
                        src_h = bass.AP(
                            tensor=qrow.tensor, offset=qrow.offset,
                            ap=[[0, 128], [1, ID]])
                        nc.scalar.dma_start(out=q_bc[:, h, :], in_=src_h)

                # bias broadcast (epilogue-only)
                psb2 = psBp.tile([128, HC], F32, tag="psbias", bufs=1)
                nc.tensor.matmul(psb2, ones1, bias_sb, start=True, stop=True)
                nc.scalar.activation(bias_bc, psb2, CPY)
                # bias-augmented transpose matrices: M_h = I65 with row 64 =
                # [bias_h | 1].  pt = s^T @ M_h then equals num + den*bias in
                # cols 0:64 (and den in col 64), so after the reciprocal
                # multiply the bias is already applied -- no separate add.
                idsl = ident[0:65, 0:65]
                ident_b = bass.AP(
                    tensor=idsl.tensor, offset=idsl.offset,
                    ap=[idsl.ap[0], [0, H], [1, 65]])
                nc.vector.tensor_copy(mmat, ident_b)
                nc.scalar.activation(mmat[64:65, :, 0:64],
                                     bias_bc[64:65, :], CPY)
                nc.gpsimd.memset(mmat[64:65, :, 64:65], 1.0)

                # main projections; expv exp + xh1 evac on ACT
                for t in range(NT):
                    ps = psA.tile([128, HC + 12], F32)
                    nc.tensor.matmul(ps, xT[:, t * 128:(t + 1) * 128], wcat,
                                     start=True, stop=True)
                    nc.scalar.activation(expv[:, t, :], ps[:, HC:HC + 8], EXP)
                    nc.scalar.activation(xh1[:, t, :, 0:64], ps[:, 0:HC], CPY)

            nc.leave_named_scope("phA", sc_a[0], False)

            # ---------------- phase B: grid + matmul accumulate ----------------
            with tc.tile_pool(name="ep_sb", bufs=1) as epsb:
                with tc.tile_pool(name="acc", bufs=1, space="PSUM") as accp:
                    acc = {}
                    for h in range(H):
                        acc[h] = accp.tile([65, ID], F32, tag=f"acc{h}",
                                           name=f"acc{h}")

                    sc_b = nc.enter_named_scope("phB", False)
                    with tc.tile_pool(name="adj", bufs=ABUFS) as adjp, \
                         tc.tile_pool(name="grid", bufs=4) as gridp:
                        for rep in range(reps):
                            for t in range(NT):
                                adjt = adjp.tile([128, ID], F16)
                                nc.sync.dma_start(out=adjt, in_=d_adjT[t])
                                g = gridp.tile([128, H, ID], F16, tag="G",
                                               bufs=GBUFS)
                                # fused grid weight, one 2x DVE op per head:
                                # g = max(Q' e1, e2) * adjT
                                for h in range(H):
                                    _maxmul(
                                        nc, out=g[:, h, :], in0=q_bc[:, h, :],
                                        in1=adjt,
                                        s0=expv[:, t, h:h + 1],
                                        s1=expv[:, t, 4 + h:5 + h])

                                first = (rep == 0 and t == 0)
                                last = (rep == reps - 1 and t == NT - 1)
                                for h in range(H):
                                    for k in range(NKD):
                                        nc.tensor.matmul(
                                            acc[h][:, k * 512:(k + 1) * 512],
                                            xh1[:, t, h, :],
                                            g[:, h, k * 512:(k + 1) * 512],
                                            start=first, stop=last)

                    nc.leave_named_scope("phB", sc_b[0], False)
                    sc_c = nc.enter_named_scope("phC", False)
                    # evacuate accumulators to SBUF, split ACT/DVE, per (h,k)
                    s_tiles = {}
                    for k in range(NKD):
                        for h in range(H):
                            s = epsb.tile([65, 512], F32, tag=f"s{h}{k}",
                                          name=f"s{h}{k}")
                            sl = acc[h][:, k * 512:(k + 1) * 512]
                            # acc0/acc1 evacuate on ACT, acc2/acc3 on DVE so
                            # the low PSUM banks (reused by the transpose
                            # tiles) free earliest
                            if h < 2:
                                nc.scalar.activation(s, sl, CPY)
                            else:
                                nc.vector.tensor_copy(s, sl)
                            s_tiles[(h, k)] = s

                # acc PSUM released here
                # ------------- phase C: transpose + divide + bias + out -------------
                with tc.tile_pool(name="ep_ps", bufs=2, space="PSUM") as epps, \
                     tc.tile_pool(name="ep_sm", bufs=2) as epsm, \
                     tc.tile_pool(name="outp", bufs=2) as outp:
                    for k in range(NKD):
                        # all four heads transposed into one PSUM tile,
                        # 128-padded blocks so no transpose crosses a bank
                        pt = epps.tile([128, H, 4, 128], F32)
                        for h in range(H):
                            for kk in range(4):
                                nc.tensor.transpose(
                                    pt[:, h, kk, 0:65],
                                    s_tiles[(h, k)][:, kk * 128:(kk + 1) * 128],
                                    mmat[:, h, :])
                        rec = epsm.tile([128, H, 4, 1], F32)
                        nc.vector.reciprocal(rec, pt[:, :, :, 64:65])
                        osb = outp.tile([128, 4, HC], F32, tag="osb", name="osb")
                        rec_rep = bass.AP(
                            tensor=rec.tensor, offset=rec.offset,
                            ap=[rec.ap[0], rec.ap[1], rec.ap[2], [0, 64]])
                        # osb[p, kk, h*64+c] = pt[p, h, kk, c] * rec[p, h, kk]
                        osb_x = bass.AP(
                            tensor=osb.tensor, offset=osb.offset,
                            ap=[osb.ap[0], [64, H], [HC, 4], [1, 64]])
                        nc.vector.tensor_tensor(
                            out=osb_x, in0=pt[:, :, :, 0:64], in1=rec_rep,
                            op=MULT)
                        # destination rows are host-permuted so partition p
                        # holds 4 consecutive output rows: one contiguous 4KB
                        # descriptor per partition
                        # two half-DMAs on different queues so the tail
                        # transfer overlaps
                        blk = d_out[k * 512:(k + 1) * 512, :]
                        for pp, eng in ((0, nc.sync), (1, nc.scalar)):
                            sub = bass.AP(
                                tensor=blk.tensor,
                                offset=blk.offset + pp * 64 * 4 * HC,
                                ap=[[4 * HC, 64], [HC, 4], [1, HC]])
                            eng.dma_start(out=sub, in_=osb[pp * 64:(pp + 1) * 64])
                    nc.leave_named_scope("phC", sc_c[0], False)

    nc.compile()
    return nc


def _get_nc(reps: int = 1):
    if reps not in _NC_CACHE:
        _NC_CACHE[reps] = build_nc(reps)
    return _NC_CACHE[reps]


def make_in_maps(x, adj, W, att_src, att_dst, bias):
    x = np.asarray(x, dtype=np.float32)
    adj = np.asarray(adj, dtype=np.float32)
    W = np.asarray(W, dtype=np.float32)
    att_src = np.asarray(att_src, dtype=np.float32)
    att_dst = np.asarray(att_dst, dtype=np.float32)
    bias = np.asarray(bias, dtype=np.float32)

    # weight prep: fold per-head attention dots into projection columns
    wa_src = np.stack([W[:, h * C:(h + 1) * C] @ att_src[h] for h in range(H)], 1)
    wa_dst = np.stack([W[:, h * C:(h + 1) * C] @ att_dst[h] for h in range(H)], 1)
    wcat = np.concatenate([W, 0.2 * wa_src, wa_src, -0.8 * wa_dst], axis=1)
    wcat = np.ascontiguousarray(wcat, dtype=np.float16)          # [F, 268]

    adjl = adj.copy()
    idx = np.arange(N)
    adjl[:, idx, idx] = 1.0

    # destination-row permutation: kernel position i' = kk*128 + p within each
    # 512-block maps to original row p*4 + kk, so the output DMA writes 4KB
    # contiguous chunks per partition
    perm = np.concatenate([kb * 512 + (np.arange(512) % 128) * 4 + np.arange(512) // 128
                           for kb in range(ID // 512)])

    in_maps = []
    for c in range(N_CORES):
        b, half = c // 2, c % 2
        # kernel source-row order: dest-half rows first, in the dest
        # permutation (so q values line up with adjT columns), then the rest
        row_order = np.concatenate([half * ID + perm,
                                    (1 - half) * ID + np.arange(ID)])
        xT = np.ascontiguousarray(x[b][row_order].T, dtype=np.float16)
        adjT = np.ascontiguousarray(
            adjl[b].T[row_order][:, half * ID:(half + 1) * ID][:, perm]
        ).astype(np.float16)       # [j(reordered), i(permuted)]
        in_maps.append({
            "xT": xT,
            "adjT": adjT.reshape(NT, 128, ID),
            "Wcat": wcat,
            "biasv": bias.reshape(1, HC),
        })
    return in_maps


def assemble(results):
    out = np.empty((B, N, HC), dtype=np.float32)
    for c in range(N_CORES):
        b, half = c // 2, c % 2
        out[b, half * ID:(half + 1) * ID, :] = results[c]["out"]
    return out


def kernel(x, adj, W, att_src, att_dst, bias):
    nc = _get_nc(1)
    in_maps = make_in_maps(x, adj, W, att_src, att_dst, bias)
    res = run_bass_kernel_spmd(nc, in_maps, list(range(N_CORES)))
    return assemble(res.results)
